# revision 4
# baseline (speedup 1.0000x reference)
"""BeatPooling segment-mean kernel for 8 Trainium2 NeuronCores.

Reference computation (per batch row):
    s = clip(bounds[:, 0], 0, T-1); e = max(s+1, min(bounds[:, 1], T))
    mean[m] = sum(frame[s_m:e_m]) / (e_m - s_m)            # via cumsum gather
    out = concat([mean, fourier(pos)], -1) @ W + b         # [M, D]

Sharding: data-parallel over B (one batch row per core). The Fourier/bias
term ff @ W[D:] + b is independent of frame data and folded on the host
into a [M, D] bias; the clamp/count index arithmetic on the tiny bounds
tensor is also host-side.

Device pipeline per core:
  1. stream the 16 MiB frame row (16 x 1 MiB DMAs)
  2. PE-transpose each [128t x 128d] tile into PSUM ([d, t] layout)
  3. DVE tensor_tensor_scan PSUM->SBUF builds the inclusive cumsum for all
     four 128-d chunks, interleaved as cs[p, t, chunk] (d=4 layout) so that
  4. ONE gpsimd ap_gather fetches all 4 chunks at the 1024 segment
     boundary columns (each indirect pool instruction costs a fixed ~27us
     sequencer-retire stall, so there must be exactly one)
  5. masked subtract (cs[e-1] - smask*cs[s-1]) -> segsum^T, then fp32
     matmuls project through W[:D]; fused recip-scale + bias on the drain.
"""

import math

import numpy as np

import concourse.bacc as bacc
import concourse.mybir as mybir
from concourse import bass_utils
from concourse.tile import TileContext

B, T, D, M = 8, 8192, 512, 512
POS_DIM = 32
P = 128
N_CORES = 8
TB = T // P            # 64 t-blocks
GROUPS = TB // 4       # 16 groups of 4 blocks (512 t each)
DC = D // P            # 4 d-chunks
MC = M // P            # 4 m-chunks

F32 = mybir.dt.float32
F32R = mybir.dt.float32r
I16 = mybir.dt.int16

_CACHED_NC = None


def _build_nc():
    nc = bacc.Bacc("TRN2", target_bir_lowering=False, debug=False,
                   num_devices=N_CORES)

    frame = nc.dram_tensor("frame", [T, D], F32, kind="ExternalInput")
    w1_in = nc.dram_tensor("w1", [D, D], F32, kind="ExternalInput")
    bias_in = nc.dram_tensor("bias", [M, D], F32, kind="ExternalInput")
    recip_in = nc.dram_tensor("recip", [P, MC], F32, kind="ExternalInput")
    idxs_in = nc.dram_tensor("idxs", [P, 2 * M // 16], I16, kind="ExternalInput")
    widx_in = nc.dram_tensor("widx", [P, 4], I16, kind="ExternalInput")
    smask_in = nc.dram_tensor("smask", [P, DC * M], F32, kind="ExternalInput")
    ident_in = nc.dram_tensor("ident", [P, P], F32, kind="ExternalInput")
    out = nc.dram_tensor("out", [M, D], F32, kind="ExternalOutput")

    add = mybir.AluOpType.add
    mult = mybir.AluOpType.mult
    bypass = mybir.AluOpType.bypass
    sub = mybir.AluOpType.subtract

    with TileContext(nc, num_cores=N_CORES) as tc:
        with (
            tc.tile_pool(name="const", bufs=1) as const,
            tc.tile_pool(name="staging", bufs=2) as staging,
            tc.tile_pool(name="psum", bufs=8, space="PSUM") as psum,
            tc.tile_pool(name="outp", bufs=2) as outp,
        ):
            # ---- long-lived tiles -------------------------------------
            w1 = [const.tile([P, D], F32, name=f"w1_{c}") for c in range(DC)]
            bias = [const.tile([P, D], F32, name=f"bias_{m}") for m in range(MC)]
            # interleaved cumsum: element (p, t*4 + c) = incl cumsum chunk c
            cs_i = const.tile([P, T * DC], F32, name="cs_i")
            cs_v = cs_i[:].rearrange("p (t c) -> p t c", c=DC)
            gath = const.tile([P, 2 * M * DC], F32, name="gath")
            gath_v = gath[:].rearrange("p (i c) -> p i c", c=DC)
            recip = const.tile([P, MC], F32, name="recip")
            idxs = const.tile([P, 2 * M // 16], I16, name="idxs")
            smask = const.tile([P, DC * M], F32, name="smask")
            ident = const.tile([P, P], F32, name="ident")

            # identity is needed by the very first transpose
            nc.sync.dma_start(ident[:], ident_in.ap())

            # warm up the gpsimd gather program early so the real gather at
            # the end doesn't pay the ~27us first-use load stall
            widx = const.tile([P, 4], I16, name="widx")
            wscr = const.tile([P, 64], F32, name="wscr")
            nc.sync.dma_start(widx[:], widx_in.ap())
            nc.gpsimd.ap_gather(
                wscr[:], ident[:], widx[:],
                channels=P, num_elems=P, d=1, num_idxs=64,
            )

            # ---- stream frame: transpose + chained scan ---------------
            frame_g = frame.ap().rearrange("(g b p) d -> g p b d", p=P, b=4)
            for g in range(GROUPS):
                stage = staging.tile([P, 4 * D], F32, name="stage", tag="stage")
                nc.sync.dma_start(
                    stage[:].rearrange("p (b d) -> p b d", b=4), frame_g[g])
                for c in range(DC):
                    ps = psum.tile([P, 512], F32, name="ps", tag="ps")
                    for b in range(4):
                        nc.tensor.transpose(
                            ps[:, b * P:(b + 1) * P],
                            stage[:, b * D + c * P: b * D + (c + 1) * P],
                            ident[:],
                        )
                    # inclusive cumsum of this 512-t piece (strided d=4
                    # interleaved output), chained via the previous piece's
                    # last column
                    nc.vector.tensor_tensor_scan(
                        out=cs_v[:, g * 512:(g + 1) * 512, c],
                        data0=ps[:],
                        data1=ident[:, 0:1].broadcast_to([P, 512]),
                        initial=(0.0 if g == 0
                                 else cs_v[:, g * 512 - 1, c:c + 1]),
                        op0=add,
                        op1=bypass,
                    )

            # constants for the tail phases (DMAs overlap streaming)
            for c in range(DC):
                nc.sync.dma_start(w1[c][:], w1_in.ap()[c * P:(c + 1) * P, :])
            for m in range(MC):
                nc.sync.dma_start(bias[m][:], bias_in.ap()[m * P:(m + 1) * P, :])
            nc.sync.dma_start(recip[:], recip_in.ap())
            nc.sync.dma_start(idxs[:], idxs_in.ap())
            nc.sync.dma_start(smask[:], smask_in.ap())

            # ---- one fused gather of all boundary columns -------------
            nc.gpsimd.ap_gather(
                gath[:],
                cs_i[:],
                idxs[:],
                channels=P,
                num_elems=T,
                d=DC,
                num_idxs=2 * M,
            )
            # segsum^T = cs[e-1] - smask * cs[s-1]  (smask kills the s==0
            # gather, whose true value is cumsum 0); s-half overwritten in
            # place
            nc.vector.tensor_tensor(
                out=gath[:, 0:M * DC],
                in0=gath[:, 0:M * DC],
                in1=smask[:],
                op=mult,
            )
            nc.vector.tensor_tensor(
                out=gath[:, 0:M * DC],
                in0=gath[:, M * DC:2 * M * DC],
                in1=gath[:, 0:M * DC],
                op=sub,
            )

            # ---- projection: psum = segsum^T . W1 ---------------------
            po = [psum.tile([P, D], F32, name=f"po_{m}", tag="ps")
                  for m in range(MC)]
            for c in range(DC):
                for m in range(MC):
                    nc.tensor.matmul(
                        po[m][:],
                        lhsT=gath_v[:, m * P:(m + 1) * P, c],
                        rhs=w1[c][:],
                        start=(c == 0),
                        stop=(c == DC - 1),
                    )

            # ---- out = recip * psum + bias ----------------------------
            for m in range(MC):
                ot = outp.tile([P, D], F32, name="ot", tag="ot")
                nc.vector.scalar_tensor_tensor(
                    out=ot[:],
                    in0=po[m][:],
                    scalar=recip[:, m:m + 1],
                    in1=bias[m][:],
                    op0=mult,
                    op1=add,
                )
                nc.sync.dma_start(out.ap()[m * P:(m + 1) * P, :], ot[:])

    nc.compile()
    return nc


def _fourier_features(pos, dim):
    half = dim // 2
    freqs = np.exp(np.linspace(0.0, math.log(1000.0), half))
    ang = pos[..., None] * freqs
    out = np.concatenate([np.sin(ang), np.cos(ang)], axis=-1)
    return out


def _host_prep(frame_emb, beat_bounds, W, b):
    """Per-core input maps (core i <- batch row i)."""
    s = np.clip(beat_bounds[:, :, 0], 0, T - 1).astype(np.int64)
    e = np.maximum(s + 1, np.minimum(beat_bounds[:, :, 1], T)).astype(np.int64)
    counts = (e - s).astype(np.float32)
    recip = (1.0 / counts).astype(np.float32)            # [B, M]

    pos = np.clip(np.arange(M, dtype=np.float64) / max(1, M - 1), 0.0, 1.0)
    ff = _fourier_features(pos, POS_DIM)                 # [M, 32]
    bias = (ff @ W[D:, :].astype(np.float64)
            + b.astype(np.float64)).astype(np.float32)   # [M, D]
    w1 = np.ascontiguousarray(W[:D, :], dtype=np.float32)
    ident = np.eye(P, dtype=np.float32)

    in_maps = []
    for i in range(B):
        # cs column j holds the inclusive cumsum at t=j, so
        #   sum over [s, e) = cs[e-1] - cs[s-1],   cs[-1] := 0 via smask
        s_idx = np.maximum(s[i] - 1, 0)
        e_idx = e[i] - 1
        idx = np.concatenate([s_idx, e_idx]).astype(np.int16)  # [2M]
        # wrapped layout: idx j lives at [16g + j%16, j//16], replicated
        # for each of the 8 gpsimd cores g
        wrapped = idx.reshape(2 * M // 16, 16).T               # [16, 2M/16]
        idxs_t = np.tile(wrapped, (8, 1)).astype(np.int16)     # [128, 2M/16]
        sm = np.repeat((s[i] > 0).astype(np.float32), DC)      # [M*DC]
        smask_t = np.tile(sm[None, :], (P, 1))                 # [128, M*DC]
        recip_t = recip[i].reshape(MC, P).T.copy()             # [P, MC]
        in_maps.append({
            "frame": np.ascontiguousarray(frame_emb[i], dtype=np.float32),
            "w1": w1,
            "bias": bias,
            "recip": recip_t,
            "idxs": idxs_t,
            "widx": np.zeros((P, 4), dtype=np.int16),
            "smask": smask_t,
            "ident": ident,
        })
    return in_maps


def get_nc():
    global _CACHED_NC
    if _CACHED_NC is None:
        _CACHED_NC = _build_nc()
    return _CACHED_NC


def kernel(frame_emb, beat_bounds, W, b, _trace=False):
    nc = get_nc()
    in_maps = _host_prep(np.asarray(frame_emb), np.asarray(beat_bounds),
                         np.asarray(W), np.asarray(b))
    res = bass_utils.run_bass_kernel_spmd(
        nc, in_maps, core_ids=list(range(N_CORES)), trace=_trace)
    out = np.stack([res.results[i]["out"] for i in range(B)], axis=0)
    if _trace:
        kernel.last_results = res
    return out

